# revision 26
# baseline (speedup 1.0000x reference)
"""Trainium2 Bass kernel: masked-LSTM readout over to_dense_batch'd graphs.

Strategy (8 NeuronCores, SPMD single program):
 - Host: per-graph lengths from sorted `index`; graphs globally sorted by
   length (desc) and dealt round-robin to 8 cores, so all cores share one
   step schedule N_t = ceil(#active_global(t)/8). Host densifies x into a
   block-major padded tensor per core (bf16).
 - Device: per time-block, DMA loads x-dense as [128 = feat + 64*(t%2),
   cols]; per step, 4 accumulating bf16 matmul pairs compute gate
   preactivations for the active column prefix. One Sigmoid ACT gives
   [f|i]; one Tanh ACT gives [T_o=tanh(z_o/2) | g~=tanh(z_g)] so the DVE
   cell update is pure tensor_tensor bf16 (2x mode):
     fc = c*f ; ig = i*g~ ; c = fc+ig   (3 TT bf16)
     tc = tanh(c)                        (ACT)
     h2 = (1+T_o)*tc = 2h               (STT; U/2 folded into stationaries)
   Each graph's final h2 is snapshotted via a predicated copy.
 - Host: gather per-core outputs (h2/2), invert the deal/sort permutation.
"""

import numpy as np
import ml_dtypes

MAXLEN = 100
B = 8192
NCORES = 8
G = B // NCORES          # graph columns per core
H = 64
F = 64
TW = 20                  # steps per time block (even)
CHUNK = 512              # matmul free-dim chunk (psum bank)

_CACHE = {}


def _build_and_compile(schedule, weights=None):
    """Build the Bass program for a given (global) schedule. Returns nc."""
    import concourse.bacc as bacc
    import concourse.mybir as mybir
    from concourse import tile

    N_t, blocks, snap = schedule  # N_t: list; blocks: [(t0, nsteps, Wb, row0)]; snap: [(lo, hi, moff)]
    bf16 = mybir.dt.bfloat16
    f32 = mybir.dt.float32
    T_end = len(N_t)
    ROWS_TOT = sum(Wb * nst // 2 for (_, nst, Wb, _) in blocks)
    MW = sum(hi - lo for pieces in snap for (_, lo, hi, _) in pieces)
    XT_W = max(Wb * nst // 2 for (_, nst, Wb, _) in blocks)

    nc = bacc.Bacc("TRN2", target_bir_lowering=False)
    xd_d = nc.dram_tensor("xd", [128, ROWS_TOT], bf16, kind="ExternalInput")
    msk_d = nc.dram_tensor("msk", [64, max(MW, 1)], mybir.dt.uint8, kind="ExternalInput")
    out_d = nc.dram_tensor("outh", [64, G], bf16, kind="ExternalOutput")

    # *_d0: gate order [f|i]/[o|g] for piece km=0; *_d1: swapped order
    # [i|f]/[g|o] for km=1, so DVE input pairs align at base partition 64
    wd, bd = {}, {}
    for k in range(2):
        wd[("fix", k)] = nc.dram_tensor(f"wfix{k}", [128, 128], bf16, kind="ExternalInput")
        wd[("ogx", k)] = nc.dram_tensor(f"wogx{k}", [128, 128], bf16, kind="ExternalInput")
        wd[("fih", k)] = nc.dram_tensor(f"wfih{k}", [64, 128], bf16, kind="ExternalInput")
        wd[("ogh", k)] = nc.dram_tensor(f"wogh{k}", [64, 128], bf16, kind="ExternalInput")
        bd[("fi", k)] = nc.dram_tensor(f"bfi{k}", [128, 1], f32, kind="ExternalInput")
        bd[("og", k)] = nc.dram_tensor(f"bog{k}", [128, 1], f32, kind="ExternalInput")
        bd[("sc", k)] = nc.dram_tensor(f"scog{k}", [128, 1], f32, kind="ExternalInput")

    Sig = mybir.ActivationFunctionType.Sigmoid
    Tanh = mybir.ActivationFunctionType.Tanh
    Mult = mybir.AluOpType.mult
    Add = mybir.AluOpType.add

    with tile.TileContext(nc) as tc:
        with tc.tile_pool(name="state", bufs=1) as sp, \
             tc.tile_pool(name="xblk", bufs=3) as xp, \
             tc.tile_pool(name="psum", bufs=2, space="PSUM") as pp:
            wfix, wogx, wfih, wogh, bfi, bog, scog = ({} for _ in range(7))
            for k in range(2):
                wfix[k] = sp.tile([128, 128], bf16, tag=f"wfix{k}", name=f"wfix{k}")
                nc.sync.dma_start(out=wfix[k], in_=wd[("fix", k)].ap())
                wogx[k] = sp.tile([128, 128], bf16, tag=f"wogx{k}", name=f"wogx{k}")
                nc.sync.dma_start(out=wogx[k], in_=wd[("ogx", k)].ap())
                wfih[k] = sp.tile([64, 128], bf16, tag=f"wfih{k}", name=f"wfih{k}")
                nc.sync.dma_start(out=wfih[k], in_=wd[("fih", k)].ap())
                wogh[k] = sp.tile([64, 128], bf16, tag=f"wogh{k}", name=f"wogh{k}")
                nc.sync.dma_start(out=wogh[k], in_=wd[("ogh", k)].ap())
                bfi[k] = sp.tile([128, 1], f32, tag=f"bfi{k}", name=f"bfi{k}")
                nc.sync.dma_start(out=bfi[k], in_=bd[("fi", k)].ap())
                bog[k] = sp.tile([128, 1], f32, tag=f"bog{k}", name=f"bog{k}")
                nc.sync.dma_start(out=bog[k], in_=bd[("og", k)].ap())
                scog[k] = sp.tile([128, 1], f32, tag=f"scog{k}", name=f"scog{k}")
                nc.sync.dma_start(out=scog[k], in_=bd[("sc", k)].ap())
            mskt = sp.tile([64, max(MW, 1)], mybir.dt.uint8)
            nc.sync.dma_start(out=mskt, in_=msk_d.ap())

            # cg/tc packed: km=0 at parts 0:64, km=1 at parts 64:128 so the
            # tanh(c) ACT covers both pieces in one 128-partition instruction
            cgt, tct = {}, {}
            for k in range(2):
                cgt[k] = sp.tile([64, CHUNK], bf16, tag=f"cg{k}", name=f"cg{k}")
                tct[k] = sp.tile([64, CHUNK], bf16, tag=f"tc{k}", name=f"tc{k}")
                nc.vector.memset(cgt[k][:, :], 0.0)

            def cgs(km, p0, p1):
                return cgt[km][:, p0:p1]

            def tcs(km, p0, p1):
                return tct[km][:, p0:p1]

            h, sfi, so, fc, ig, outh = ({} for _ in range(6))
            for k in range(2):
                h[k] = sp.tile([64, CHUNK], bf16, tag=f"h{k}", name=f"h{k}")
                sfi[k] = sp.tile([128, CHUNK], bf16, tag=f"sfi{k}", name=f"sfi{k}")
                so[k] = sp.tile([128, CHUNK], bf16, tag=f"so{k}", name=f"so{k}")
                fc[k] = sp.tile([64, CHUNK], bf16, tag=f"fc{k}", name=f"fc{k}")
                ig[k] = sp.tile([64, CHUNK], bf16, tag=f"ig{k}", name=f"ig{k}")
                outh[k] = sp.tile([64, CHUNK], bf16, tag=f"oh{k}", name=f"oh{k}")
                nc.vector.memset(h[k][:, :], 0.0)
                nc.vector.memset(outh[k][:, :], 0.0)

            for (t0, nsteps, Wb, row0) in blocks:
                rows_b = Wb * nsteps // 2
                xt = xp.tile([128, XT_W], bf16, tag="xt")
                nc.sync.dma_start(
                    out=xt[:, 0:rows_b], in_=xd_d.ap()[:, row0:row0 + rows_b])

                for ts in range(nsteps):
                    t = t0 + ts
                    n = N_t[t]
                    if n == 0:
                        continue
                    par = ts % 2
                    # work items: (psum_tag, state_tile, p0, p1); tail steps
                    # split the lone chunk into two pieces on separate psum
                    # banks so their ACT/DVE chains can interleave
                    # mms/acts: (psum_tag, state_tile, psum_col0, p0, p1)
                    # dve: (state_tile, p0, p1) — split for engine pipelining
                    if n > CHUNK:
                        work = [(0, 0, 0, 0, CHUNK), (1, 1, 0, 0, n - CHUNK)]
                        dve = [(0, 0, CHUNK), (1, 0, n - CHUNK)]
                    elif n >= 128:
                        m = (n // 2 + 1) & ~1
                        work = [(0, 0, 0, 0, m), (1, 0, 0, m, n)]
                        dve = [(0, 0, m), (0, m, n)]
                    else:
                        work = [(0, 0, 0, 0, n)]
                        dve = [(0, 0, n)]
                    acts = work
                    fi_ps, og_ps = {}, {}
                    # x-side matmuls first (h-independent): the PE FIFO runs
                    # them during the previous step's elementwise phase, so
                    # only the h-side matmuls sit on the recurrence chain
                    for (kt, km, q0, p0, p1) in work:
                        w = p1 - p0
                        c0 = CHUNK * km + p0
                        if kt not in fi_ps:
                            fi_ps[kt] = pp.tile([128, CHUNK], f32, tag=f"fi{kt}", name=f"fi{kt}")
                            og_ps[kt] = pp.tile([128, CHUNK], f32, tag=f"og{kt}", name=f"og{kt}")
                        xs = xt[par * 64:(par + 1) * 64,
                                ts // 2 * Wb + c0:
                                ts // 2 * Wb + c0 + w]
                        nc.tensor.matmul(out=fi_ps[kt][:, q0:q0 + w],
                                         lhsT=wfix[km][par * 64:(par + 1) * 64, :],
                                         rhs=xs, start=True, stop=False)
                        nc.tensor.matmul(out=og_ps[kt][:, q0:q0 + w],
                                         lhsT=wogx[km][par * 64:(par + 1) * 64, :],
                                         rhs=xs, start=True, stop=False)
                    for (kt, km, q0, p0, p1) in work:
                        w = p1 - p0
                        nc.tensor.matmul(out=fi_ps[kt][:, q0:q0 + w],
                                         lhsT=wfih[km][:, :],
                                         rhs=h[km][:, p0:p1], start=False, stop=True)
                        nc.tensor.matmul(out=og_ps[kt][:, q0:q0 + w],
                                         lhsT=wogh[km][:, :],
                                         rhs=h[km][:, p0:p1], start=False, stop=True)
                    for (kt, km, q0, a0, a1) in acts:
                        w = a1 - a0
                        nc.scalar.activation(out=sfi[km][:, a0:a1], in_=fi_ps[kt][:, q0:q0 + w],
                                             func=Sig, bias=bfi[km][:, :])
                        nc.scalar.activation(out=so[km][:, a0:a1], in_=og_ps[kt][:, q0:q0 + w],
                                             func=Tanh, bias=bog[km][:, :], scale=scog[km][:, :])
                    for (km, p0, p1) in dve:
                        fsl = slice(0, 64)      # f / T_o half
                        isl = slice(64, 128)    # i / g~ half
                        nc.vector.tensor_tensor(
                            out=fc[km][:, p0:p1], in0=cgs(km, p0, p1),
                            in1=sfi[km][fsl, p0:p1], op=Mult)
                        nc.vector.tensor_tensor(
                            out=ig[km][:, p0:p1], in0=sfi[km][isl, p0:p1],
                            in1=so[km][isl, p0:p1], op=Mult)
                        nc.vector.tensor_tensor(
                            out=cgs(km, p0, p1), in0=fc[km][:, p0:p1],
                            in1=ig[km][:, p0:p1], op=Add)
                    for (km, p0, p1) in dve:
                        nc.scalar.activation(out=tcs(km, p0, p1),
                                             in_=cgs(km, p0, p1), func=Tanh)
                        nc.vector.scalar_tensor_tensor(
                            out=h[km][:, p0:p1], in0=so[km][0:64, p0:p1], scalar=1.0,
                            in1=tcs(km, p0, p1), op0=Add, op1=Mult)
                    for (kk, lo, hi, moff) in snap[t]:
                        nc.vector.copy_predicated(
                            out=outh[kk][:, lo:hi],
                            mask=mskt[:, moff:moff + (hi - lo)],
                            data=h[kk][:, lo:hi])

            nc.sync.dma_start(out=out_d.ap()[:, 0:CHUNK], in_=outh[0][:, :])
            nc.sync.dma_start(out=out_d.ap()[:, CHUNK:G], in_=outh[1][:, :])
    nc.compile()
    return nc


def _plan(lens):
    """Global schedule from capped lengths [B]. Returns (order, schedule helpers)."""
    order = np.argsort(-lens, kind="stable")
    lens_sorted = lens[order]
    T_end = int(lens_sorted.max())
    # per-core sorted lengths: core c, col j -> lens_sorted[8j + c]
    len_c = lens_sorted.reshape(G, NCORES).T  # [NCORES, G]
    # n_c(t) = #cols with len > t
    t_ax = np.arange(T_end + 1)
    n_c = (len_c[:, :, None] > t_ax[None, None, :]).sum(axis=1)  # [NCORES, T_end+1]
    N_t = n_c.max(axis=0)  # [T_end+1]; N_t[T_end] == 0
    # time blocks
    blocks = []
    row0 = 0
    t0 = 0
    while t0 < T_end:
        nsteps = min(TW, T_end - t0)
        if nsteps % 2:
            nsteps += 1  # keep even; schedule N_t beyond T_end is 0-pad
        Wb = int(np.ceil(N_t[t0] / 16) * 16)
        blocks.append((t0, nsteps, Wb, row0))
        row0 += Wb * nsteps // 2
        t0 += nsteps
    # snapshot ranges + masks
    snap = []
    moff = 0
    mask_cols = []
    for t in range(T_end):
        nt1 = n_c[:, t + 1] if t + 1 <= T_end else np.zeros(NCORES, np.int64)
        lo = int(nt1.min())
        hi = int(n_c[:, t].max())
        pieces = []
        if hi > lo:
            m = np.zeros((NCORES, hi - lo), np.uint8)
            for c in range(NCORES):
                a, b_ = int(nt1[c]), int(n_c[c, t])
                m[c, max(a - lo, 0):max(b_ - lo, 0)] = 1
            mask_cols.append(m)
            for k in range(2):
                plo = max(lo, 512 * k)
                phi = min(hi, 512 * (k + 1))
                if phi > plo:
                    pieces.append((k, plo - 512 * k, phi - 512 * k,
                                   moff + (plo - lo)))
            moff += hi - lo
        snap.append(pieces)
    masks = (np.concatenate(mask_cols, axis=1) if mask_cols
             else np.zeros((NCORES, 1), np.uint8))
    # pad schedule for block overhang (nsteps even rounding)
    N_pad = list(N_t[:T_end])
    total_steps = sum(ns for (_, ns, _, _) in blocks)
    while len(N_pad) < total_steps:
        N_pad.append(0)
        snap.append([])
    # drop zero-width steps from the tail of the schedule
    sched_N = [int(x) for x in N_pad]
    return order, len_c, n_c, sched_N, blocks, snap, masks


LAST_RUN = {}


def _install_ntff_shim():
    import sys, types
    if "antenv.axon_hooks" in sys.modules:
        return
    try:
        from trn_agent_boot.trn_boot import _ntff_profile_via_ctypes
        hook = _ntff_profile_via_ctypes("/opt/axon/libaxon_pjrt.so")
    except Exception:
        hook = None
    m = types.ModuleType("antenv.axon_hooks")
    m._hook = hook
    m.get_axon_ntff_profile_hook = lambda: m._hook
    m.set_axon_ntff_profile_hook = lambda h: setattr(m, "_hook", h)
    sys.modules["antenv.axon_hooks"] = m


def kernel(x, W_ih, W_hh, b_ih, b_hh, index, dim_size, _trace=False):
    from concourse.bass_utils import run_bass_kernel_spmd
    if _trace:
        import concourse.bass_utils as _bu
        _install_ntff_shim()
        _bu.upload_artifacts = lambda d: d  # no bucket in this container

    x = np.asarray(x)
    index = np.asarray(index).astype(np.int64)
    W_ih = np.asarray(W_ih, dtype=np.float32)
    W_hh = np.asarray(W_hh, dtype=np.float32)
    b_ih = np.asarray(b_ih, dtype=np.float32)
    b_hh = np.asarray(b_hh, dtype=np.float32)

    assert int(dim_size) == B, f"kernel hardcodes B={B}, got dim_size={int(dim_size)}"
    counts = np.bincount(index, minlength=B).astype(np.int64)
    offsets = np.concatenate([[0], np.cumsum(counts)[:-1]])
    lens = np.minimum(counts, MAXLEN)

    order, len_c, n_c, N_t, blocks, snap, masks = _plan(lens)

    # --- weights (torch gate order i,f,g,o -> ours f,i / o,g) ---
    b = (b_ih + b_hh).reshape(4, H)
    Wi, Wf, Wg, Wo = W_ih.reshape(4, H, F)
    Ui, Uf, Ug, Uo = W_hh.reshape(4, H, H)
    bf16 = ml_dtypes.bfloat16

    # ih stationaries duplicated at both parity halves (x-slices alternate
    # partition halves); hh stationaries at parts 0:64 (h2 lives there),
    # halved because the recurrent rhs is h2 = 2h. Piece km=1 uses swapped
    # gate order ([i|f], [g|o]) so DVE input pairs share a base partition.
    # og ACT: tanh(scale*z + bias): T_o = tanh(0.5 z_o + 0.5 b_o),
    # g~ = tanh(z_g + b_g)
    ones = np.ones(64)
    wm = {}
    for k in range(2):
        fi = [Wf.T, Wi.T]
        og = [Wo.T, Wg.T]
        fih = [Uf.T, Ui.T]
        ogh = [Uo.T, Ug.T]
        wm[f"wfix{k}"] = np.concatenate([np.concatenate(fi, 1)] * 2, 0).astype(bf16)
        wm[f"wogx{k}"] = np.concatenate([np.concatenate(og, 1)] * 2, 0).astype(bf16)
        wm[f"wfih{k}"] = (0.5 * np.concatenate(fih, 1)).astype(bf16)
        wm[f"wogh{k}"] = (0.5 * np.concatenate(ogh, 1)).astype(bf16)
        bfi_p = [b[1], b[0]]
        bog_p = [0.5 * b[3], b[2]]
        sc_p = [0.5 * ones, ones]
        wm[f"bfi{k}"] = np.concatenate(bfi_p).reshape(128, 1).astype(np.float32)
        wm[f"bog{k}"] = np.concatenate(bog_p).reshape(128, 1).astype(np.float32)
        wm[f"scog{k}"] = np.concatenate(sc_p).reshape(128, 1).astype(np.float32)

    # --- per-core dense input (block-major) ---
    x_bf = x.astype(bf16)
    T_end = len(N_t)
    in_maps = []
    for c in range(NCORES):
        gids = order[np.arange(G) * NCORES + c]     # col j -> graph id
        lens_cj = len_c[c]                          # [G]
        offs_cj = offsets[gids]
        parts = []
        for (t0, nsteps, Wb, row0) in blocks:
            tsl = np.arange(t0, t0 + nsteps)
            node = offs_cj[:Wb, None] + tsl[None, :]             # [Wb, nsteps]
            valid = tsl[None, :] < lens_cj[:Wb, None]
            node = np.clip(node, 0, x.shape[0] - 1)
            blk = np.where(valid[:, :, None], x_bf[node], bf16(0))  # [Wb, nsteps, 64]
            # time-major rows: row r = taupair*Wb + g  -> per-step rhs contiguous
            blk = blk.reshape(Wb, nsteps // 2, 128).transpose(1, 0, 2)
            parts.append(blk.reshape(nsteps // 2 * Wb, 128))
        xd = np.ascontiguousarray(np.concatenate(parts, axis=0).T)
        msk = np.ascontiguousarray(
            np.broadcast_to(masks[c][None, :], (64, masks.shape[1])))
        in_maps.append({"xd": xd, "msk": msk, **wm})

    import hashlib
    key = hashlib.sha1((repr((tuple(N_t), tuple(blocks), repr(snap))) +
                        repr(sorted(wm))).encode()
                       + W_ih.tobytes() + W_hh.tobytes()
                       + b_ih.tobytes() + b_hh.tobytes()).hexdigest()
    if key not in _CACHE:
        _CACHE[key] = _build_and_compile((N_t, blocks, snap), None)
    nc = _CACHE[key]

    res = run_bass_kernel_spmd(nc, in_maps, core_ids=list(range(NCORES)),
                               trace=_trace)
    LAST_RUN["res"] = res

    out = np.zeros((B, H), np.float32)
    for c in range(NCORES):
        hT = res.results[c]["outh"].astype(np.float32)  # [64, G] (= 2h)
        gids = order[np.arange(G) * NCORES + c]
        out[gids] = 0.5 * hT.T
    return out


# revision 31
# speedup vs baseline: 1.0284x; 1.0284x over previous
"""Trainium2 Bass kernel: masked-LSTM readout over to_dense_batch'd graphs.

Strategy (8 NeuronCores, SPMD single program):
 - Host: per-graph lengths from sorted `index`; graphs globally sorted by
   length (desc) and dealt round-robin to 8 cores, so all cores share one
   step schedule N_t = ceil(#active_global(t)/8). Host densifies x into a
   block-major padded tensor per core (bf16).
 - Device: per time-block, DMA loads x-dense as [128 = feat + 64*(t%2),
   cols]; per step, 4 accumulating bf16 matmul pairs compute gate
   preactivations for the active column prefix. One Sigmoid ACT gives
   [f|i]; one Tanh ACT gives [T_o=tanh(z_o/2) | g~=tanh(z_g)] so the DVE
   cell update is pure tensor_tensor bf16 (2x mode):
     fc = c*f ; ig = i*g~ ; c = fc+ig   (3 TT bf16)
     tc = tanh(c)                        (ACT)
     h2 = (1+T_o)*tc = 2h               (STT; U/2 folded into stationaries)
   Each graph's final h2 is snapshotted via a predicated copy.
 - Host: gather per-core outputs (h2/2), invert the deal/sort permutation.
"""

import numpy as np
import ml_dtypes

MAXLEN = 100
B = 8192
NCORES = 8
G = B // NCORES          # graph columns per core
H = 64
F = 64
TW = 20                  # steps per time block (even)
CHUNK = 512              # matmul free-dim chunk (psum bank)

_CACHE = {}


def _build_and_compile(schedule, weights=None):
    """Build the Bass program for a given (global) schedule. Returns nc."""
    import concourse.bacc as bacc
    import concourse.mybir as mybir
    from concourse import tile

    N_t, blocks, snap = schedule  # N_t: list; blocks: [(t0, nsteps, Wb, row0)]; snap: [(lo, hi, moff)]
    bf16 = mybir.dt.bfloat16
    f32 = mybir.dt.float32
    T_end = len(N_t)
    ROWS_TOT = sum(Wb * nst // 2 for (_, nst, Wb, _) in blocks)
    MW = sum(hi - lo for pieces in snap for (_, lo, hi, _) in pieces)
    XT_W = max(Wb * nst // 2 for (_, nst, Wb, _) in blocks)

    nc = bacc.Bacc("TRN2", target_bir_lowering=False)
    xd_d = nc.dram_tensor("xd", [128, ROWS_TOT], bf16, kind="ExternalInput")
    msk_d = nc.dram_tensor("msk", [64, max(MW, 1)], mybir.dt.uint8, kind="ExternalInput")
    out_d = nc.dram_tensor("outh", [64, G], bf16, kind="ExternalOutput")

    # *_d0: gate order [f|i]/[o|g] for piece km=0; *_d1: swapped order
    # [i|f]/[g|o] for km=1, so DVE input pairs align at base partition 64
    wd, bd = {}, {}
    for k in range(2):
        wd[("fix", k)] = nc.dram_tensor(f"wfix{k}", [128, 128], bf16, kind="ExternalInput")
        wd[("ogx", k)] = nc.dram_tensor(f"wogx{k}", [128, 128], bf16, kind="ExternalInput")
        wd[("fih", k)] = nc.dram_tensor(f"wfih{k}", [64, 128], bf16, kind="ExternalInput")
        wd[("ogh", k)] = nc.dram_tensor(f"wogh{k}", [64, 128], bf16, kind="ExternalInput")
        bd[("fi", k)] = nc.dram_tensor(f"bfi{k}", [128, 1], f32, kind="ExternalInput")
        bd[("og", k)] = nc.dram_tensor(f"bog{k}", [128, 1], f32, kind="ExternalInput")
        bd[("sc", k)] = nc.dram_tensor(f"scog{k}", [128, 1], f32, kind="ExternalInput")

    Sig = mybir.ActivationFunctionType.Sigmoid
    Tanh = mybir.ActivationFunctionType.Tanh
    Mult = mybir.AluOpType.mult
    Add = mybir.AluOpType.add

    with tile.TileContext(nc) as tc:
        with tc.tile_pool(name="state", bufs=1) as sp, \
             tc.tile_pool(name="xblk", bufs=3) as xp, \
             tc.tile_pool(name="psum", bufs=2, space="PSUM") as pp:
            wfix, wogx, wfih, wogh, bfi, bog, scog = ({} for _ in range(7))
            for k in range(2):
                wfix[k] = sp.tile([128, 128], bf16, tag=f"wfix{k}", name=f"wfix{k}")
                nc.sync.dma_start(out=wfix[k], in_=wd[("fix", k)].ap())
                wogx[k] = sp.tile([128, 128], bf16, tag=f"wogx{k}", name=f"wogx{k}")
                nc.sync.dma_start(out=wogx[k], in_=wd[("ogx", k)].ap())
                wfih[k] = sp.tile([64, 128], bf16, tag=f"wfih{k}", name=f"wfih{k}")
                nc.sync.dma_start(out=wfih[k], in_=wd[("fih", k)].ap())
                wogh[k] = sp.tile([64, 128], bf16, tag=f"wogh{k}", name=f"wogh{k}")
                nc.sync.dma_start(out=wogh[k], in_=wd[("ogh", k)].ap())
                bfi[k] = sp.tile([128, 1], f32, tag=f"bfi{k}", name=f"bfi{k}")
                nc.sync.dma_start(out=bfi[k], in_=bd[("fi", k)].ap())
                bog[k] = sp.tile([128, 1], f32, tag=f"bog{k}", name=f"bog{k}")
                nc.sync.dma_start(out=bog[k], in_=bd[("og", k)].ap())
                scog[k] = sp.tile([128, 1], f32, tag=f"scog{k}", name=f"scog{k}")
                nc.sync.dma_start(out=scog[k], in_=bd[("sc", k)].ap())
            mskt = sp.tile([64, max(MW, 1)], mybir.dt.uint8)
            nc.sync.dma_start(out=mskt, in_=msk_d.ap())

            # cg/tc packed: km=0 at parts 0:64, km=1 at parts 64:128 so the
            # tanh(c) ACT covers both pieces in one 128-partition instruction
            # single wide state tiles (all pieces at km=0, columns 0..1024)
            # so every step can split at n/2 for width-balanced pieces
            cgt, tct = {}, {}
            cgt[0] = sp.tile([64, 2 * CHUNK], bf16, name="cg0")
            tct[0] = sp.tile([64, 2 * CHUNK], bf16, name="tc0")
            nc.vector.memset(cgt[0][:, :], 0.0)

            def cgs(km, p0, p1):
                return cgt[km][:, p0:p1]

            def tcs(km, p0, p1):
                return tct[km][:, p0:p1]

            h, sfi, so, fc, ig, outh = ({} for _ in range(6))
            h[0] = sp.tile([64, 2 * CHUNK], bf16, name="h0")
            sfi[0] = sp.tile([128, 2 * CHUNK], bf16, name="sfi0")
            so[0] = sp.tile([128, 2 * CHUNK], bf16, name="so0")
            fc[0] = sp.tile([64, 2 * CHUNK], bf16, name="fc0")
            ig[0] = sp.tile([64, 2 * CHUNK], bf16, name="ig0")
            outh[0] = sp.tile([64, 2 * CHUNK], bf16, name="oh0")
            nc.vector.memset(h[0][:, :], 0.0)
            nc.vector.memset(outh[0][:, :], 0.0)

            for (t0, nsteps, Wb, row0) in blocks:
                rows_b = Wb * nsteps // 2
                xt = xp.tile([128, XT_W], bf16, tag="xt")
                nc.sync.dma_start(
                    out=xt[:, 0:rows_b], in_=xd_d.ap()[:, row0:row0 + rows_b])

                for ts in range(nsteps):
                    t = t0 + ts
                    n = N_t[t]
                    if n == 0:
                        continue
                    par = ts % 2
                    # work items: (psum_tag, state_tile, p0, p1); tail steps
                    # split the lone chunk into two pieces on separate psum
                    # banks so their ACT/DVE chains can interleave
                    # mms/acts: (psum_tag, state_tile, psum_col0, p0, p1)
                    # dve: (state_tile, p0, p1) — split for engine pipelining
                    if n > CHUNK:
                        m = (n // 2 + 1) & ~1
                        work = [(0, 0, 0, 0, m), (1, 0, 0, m, n)]
                        dve = [(0, 0, m), (0, m, n)]
                    elif n >= 128:
                        m = (n // 2 + 1) & ~1
                        work = [(0, 0, 0, 0, m), (1, 0, 0, m, n)]
                        dve = [(0, 0, m), (0, m, n)]
                    else:
                        work = [(0, 0, 0, 0, n)]
                        dve = [(0, 0, n)]
                    acts = work
                    fi_ps, og_ps = {}, {}
                    # x-side matmuls first (h-independent): the PE FIFO runs
                    # them during the previous step's elementwise phase, so
                    # only the h-side matmuls sit on the recurrence chain
                    for (kt, km, q0, p0, p1) in work:
                        w = p1 - p0
                        c0 = CHUNK * km + p0
                        if kt not in fi_ps:
                            fi_ps[kt] = pp.tile([128, CHUNK], f32, tag=f"fi{kt}", name=f"fi{kt}")
                            og_ps[kt] = pp.tile([128, CHUNK], f32, tag=f"og{kt}", name=f"og{kt}")
                        xs = xt[par * 64:(par + 1) * 64,
                                ts // 2 * Wb + c0:
                                ts // 2 * Wb + c0 + w]
                        nc.tensor.matmul(out=fi_ps[kt][:, q0:q0 + w],
                                         lhsT=wfix[km][par * 64:(par + 1) * 64, :],
                                         rhs=xs, start=True, stop=False)
                        nc.tensor.matmul(out=og_ps[kt][:, q0:q0 + w],
                                         lhsT=wogx[km][par * 64:(par + 1) * 64, :],
                                         rhs=xs, start=True, stop=False)
                    for (kt, km, q0, p0, p1) in work:
                        w = p1 - p0
                        nc.tensor.matmul(out=fi_ps[kt][:, q0:q0 + w],
                                         lhsT=wfih[km][:, :],
                                         rhs=h[km][:, p0:p1], start=False, stop=True)
                        nc.tensor.matmul(out=og_ps[kt][:, q0:q0 + w],
                                         lhsT=wogh[km][:, :],
                                         rhs=h[km][:, p0:p1], start=False, stop=True)
                    for (kt, km, q0, a0, a1) in acts:
                        w = a1 - a0
                        nc.scalar.activation(out=sfi[km][:, a0:a1], in_=fi_ps[kt][:, q0:q0 + w],
                                             func=Sig, bias=bfi[km][:, :])
                        nc.scalar.activation(out=so[km][:, a0:a1], in_=og_ps[kt][:, q0:q0 + w],
                                             func=Tanh, bias=bog[km][:, :], scale=scog[km][:, :])
                    for (km, p0, p1) in dve:
                        fsl = slice(0, 64)      # f / T_o half
                        isl = slice(64, 128)    # i / g~ half
                        nc.vector.tensor_tensor(
                            out=fc[km][:, p0:p1], in0=cgs(km, p0, p1),
                            in1=sfi[km][fsl, p0:p1], op=Mult)
                        nc.vector.tensor_tensor(
                            out=ig[km][:, p0:p1], in0=sfi[km][isl, p0:p1],
                            in1=so[km][isl, p0:p1], op=Mult)
                        nc.vector.tensor_tensor(
                            out=cgs(km, p0, p1), in0=fc[km][:, p0:p1],
                            in1=ig[km][:, p0:p1], op=Add)
                    for (km, p0, p1) in dve:
                        nc.scalar.activation(out=tcs(km, p0, p1),
                                             in_=cgs(km, p0, p1), func=Tanh)
                        nc.vector.scalar_tensor_tensor(
                            out=h[km][:, p0:p1], in0=so[km][0:64, p0:p1], scalar=1.0,
                            in1=tcs(km, p0, p1), op0=Add, op1=Mult)
                    for (kk, lo, hi, moff) in snap[t]:
                        glo, ghi = CHUNK * kk + lo, CHUNK * kk + hi
                        nc.vector.copy_predicated(
                            out=outh[0][:, glo:ghi],
                            mask=mskt[:, moff:moff + (hi - lo)],
                            data=h[0][:, glo:ghi])

            nc.sync.dma_start(out=out_d.ap()[:, :], in_=outh[0][:, 0:G])
    nc.compile()
    return nc


def _plan(lens):
    """Global schedule from capped lengths [B]. Returns (order, schedule helpers)."""
    order = np.argsort(-lens, kind="stable")
    lens_sorted = lens[order]
    T_end = int(lens_sorted.max())
    # per-core sorted lengths: core c, col j -> lens_sorted[8j + c]
    len_c = lens_sorted.reshape(G, NCORES).T  # [NCORES, G]
    # n_c(t) = #cols with len > t
    t_ax = np.arange(T_end + 1)
    n_c = (len_c[:, :, None] > t_ax[None, None, :]).sum(axis=1)  # [NCORES, T_end+1]
    N_t = n_c.max(axis=0)  # [T_end+1]; N_t[T_end] == 0
    # time blocks
    blocks = []
    row0 = 0
    t0 = 0
    while t0 < T_end:
        nsteps = min(TW, T_end - t0)
        if nsteps % 2:
            nsteps += 1  # keep even; schedule N_t beyond T_end is 0-pad
        Wb = int(np.ceil(N_t[t0] / 16) * 16)
        blocks.append((t0, nsteps, Wb, row0))
        row0 += Wb * nsteps // 2
        t0 += nsteps
    # snapshot ranges + masks
    snap = []
    moff = 0
    mask_cols = []
    for t in range(T_end):
        nt1 = n_c[:, t + 1] if t + 1 <= T_end else np.zeros(NCORES, np.int64)
        lo = int(nt1.min())
        hi = int(n_c[:, t].max())
        pieces = []
        if hi > lo:
            m = np.zeros((NCORES, hi - lo), np.uint8)
            for c in range(NCORES):
                a, b_ = int(nt1[c]), int(n_c[c, t])
                m[c, max(a - lo, 0):max(b_ - lo, 0)] = 1
            mask_cols.append(m)
            for k in range(2):
                plo = max(lo, 512 * k)
                phi = min(hi, 512 * (k + 1))
                if phi > plo:
                    pieces.append((k, plo - 512 * k, phi - 512 * k,
                                   moff + (plo - lo)))
            moff += hi - lo
        snap.append(pieces)
    masks = (np.concatenate(mask_cols, axis=1) if mask_cols
             else np.zeros((NCORES, 1), np.uint8))
    # pad schedule for block overhang (nsteps even rounding)
    N_pad = list(N_t[:T_end])
    total_steps = sum(ns for (_, ns, _, _) in blocks)
    while len(N_pad) < total_steps:
        N_pad.append(0)
        snap.append([])
    # drop zero-width steps from the tail of the schedule
    sched_N = [int(x) for x in N_pad]
    return order, len_c, n_c, sched_N, blocks, snap, masks


LAST_RUN = {}


def _install_ntff_shim():
    import sys, types
    if "antenv.axon_hooks" in sys.modules:
        return
    try:
        from trn_agent_boot.trn_boot import _ntff_profile_via_ctypes
        hook = _ntff_profile_via_ctypes("/opt/axon/libaxon_pjrt.so")
    except Exception:
        hook = None
    m = types.ModuleType("antenv.axon_hooks")
    m._hook = hook
    m.get_axon_ntff_profile_hook = lambda: m._hook
    m.set_axon_ntff_profile_hook = lambda h: setattr(m, "_hook", h)
    sys.modules["antenv.axon_hooks"] = m


def kernel(x, W_ih, W_hh, b_ih, b_hh, index, dim_size, _trace=False):
    from concourse.bass_utils import run_bass_kernel_spmd
    if _trace:
        import concourse.bass_utils as _bu
        _install_ntff_shim()
        _bu.upload_artifacts = lambda d: d  # no bucket in this container

    x = np.asarray(x)
    index = np.asarray(index).astype(np.int64)
    W_ih = np.asarray(W_ih, dtype=np.float32)
    W_hh = np.asarray(W_hh, dtype=np.float32)
    b_ih = np.asarray(b_ih, dtype=np.float32)
    b_hh = np.asarray(b_hh, dtype=np.float32)

    assert int(dim_size) == B, f"kernel hardcodes B={B}, got dim_size={int(dim_size)}"
    counts = np.bincount(index, minlength=B).astype(np.int64)
    offsets = np.concatenate([[0], np.cumsum(counts)[:-1]])
    lens = np.minimum(counts, MAXLEN)

    order, len_c, n_c, N_t, blocks, snap, masks = _plan(lens)

    # --- weights (torch gate order i,f,g,o -> ours f,i / o,g) ---
    b = (b_ih + b_hh).reshape(4, H)
    Wi, Wf, Wg, Wo = W_ih.reshape(4, H, F)
    Ui, Uf, Ug, Uo = W_hh.reshape(4, H, H)
    bf16 = ml_dtypes.bfloat16

    # ih stationaries duplicated at both parity halves (x-slices alternate
    # partition halves); hh stationaries at parts 0:64 (h2 lives there),
    # halved because the recurrent rhs is h2 = 2h. Piece km=1 uses swapped
    # gate order ([i|f], [g|o]) so DVE input pairs share a base partition.
    # og ACT: tanh(scale*z + bias): T_o = tanh(0.5 z_o + 0.5 b_o),
    # g~ = tanh(z_g + b_g)
    ones = np.ones(64)
    wm = {}
    for k in range(2):
        fi = [Wf.T, Wi.T]
        og = [Wo.T, Wg.T]
        fih = [Uf.T, Ui.T]
        ogh = [Uo.T, Ug.T]
        wm[f"wfix{k}"] = np.concatenate([np.concatenate(fi, 1)] * 2, 0).astype(bf16)
        wm[f"wogx{k}"] = np.concatenate([np.concatenate(og, 1)] * 2, 0).astype(bf16)
        wm[f"wfih{k}"] = (0.5 * np.concatenate(fih, 1)).astype(bf16)
        wm[f"wogh{k}"] = (0.5 * np.concatenate(ogh, 1)).astype(bf16)
        bfi_p = [b[1], b[0]]
        bog_p = [0.5 * b[3], b[2]]
        sc_p = [0.5 * ones, ones]
        wm[f"bfi{k}"] = np.concatenate(bfi_p).reshape(128, 1).astype(np.float32)
        wm[f"bog{k}"] = np.concatenate(bog_p).reshape(128, 1).astype(np.float32)
        wm[f"scog{k}"] = np.concatenate(sc_p).reshape(128, 1).astype(np.float32)

    # --- per-core dense input (block-major) ---
    x_bf = x.astype(bf16)
    T_end = len(N_t)
    in_maps = []
    for c in range(NCORES):
        gids = order[np.arange(G) * NCORES + c]     # col j -> graph id
        lens_cj = len_c[c]                          # [G]
        offs_cj = offsets[gids]
        parts = []
        for (t0, nsteps, Wb, row0) in blocks:
            tsl = np.arange(t0, t0 + nsteps)
            node = offs_cj[:Wb, None] + tsl[None, :]             # [Wb, nsteps]
            valid = tsl[None, :] < lens_cj[:Wb, None]
            node = np.clip(node, 0, x.shape[0] - 1)
            blk = np.where(valid[:, :, None], x_bf[node], bf16(0))  # [Wb, nsteps, 64]
            # time-major rows: row r = taupair*Wb + g  -> per-step rhs contiguous
            blk = blk.reshape(Wb, nsteps // 2, 128).transpose(1, 0, 2)
            parts.append(blk.reshape(nsteps // 2 * Wb, 128))
        xd = np.ascontiguousarray(np.concatenate(parts, axis=0).T)
        msk = np.ascontiguousarray(
            np.broadcast_to(masks[c][None, :], (64, masks.shape[1])))
        in_maps.append({"xd": xd, "msk": msk, **wm})

    import hashlib
    key = hashlib.sha1((repr((tuple(N_t), tuple(blocks), repr(snap))) +
                        repr(sorted(wm))).encode()
                       + W_ih.tobytes() + W_hh.tobytes()
                       + b_ih.tobytes() + b_hh.tobytes()).hexdigest()
    if key not in _CACHE:
        _CACHE[key] = _build_and_compile((N_t, blocks, snap), None)
    nc = _CACHE[key]

    res = run_bass_kernel_spmd(nc, in_maps, core_ids=list(range(NCORES)),
                               trace=_trace)
    LAST_RUN["res"] = res

    out = np.zeros((B, H), np.float32)
    for c in range(NCORES):
        hT = res.results[c]["outh"].astype(np.float32)  # [64, G] (= 2h)
        gids = order[np.arange(G) * NCORES + c]
        out[gids] = 0.5 * hT.T
    return out
